# revision 60
# baseline (speedup 1.0000x reference)
"""Trainium2 Bass kernel for the DPAAUser3D segment-reduce problem.

Computes, for x[B=2,C=8,D=H=W=128] and attentions[B,C,512,1]:
  onehot = one_hot(argmax_c x)                      (per-voxel channel argmax)
  adj    = avgpool_8x8x8(onehot)                    ([B,C,16,16,16], = counts/512)
  corr[b,c,D,H,W] = att[b,c,(D//16*8+H//16)*8+W//16] * adj[b,c,D%16,H%16,W%16]
  out1   = x * (1+corr)^2
  out2   = corr

Sharding: data-parallel over D (16 slices per core, 8 cores); per-core
pooled counts are AllGathered per (batch, channel-quad).

v4: fp16 end-to-end. The host rounds x to fp16 and nudges so the fp16
argmax one-hot EXACTLY matches the f32 argmax one-hot (non-argmax
channels that round to >= the argmax value are clamped one fp16 ulp
below it; perturbation <= 1 ulp ~ 5e-4 rel). Consequences:
  - x HBM load traffic halves (fp16 instead of f32)
  - every DVE tensor_tensor runs in 2x perf mode (16-bit packed)
  - argmax/one-hot is numerically EXACT vs the reference
Pipeline per core (single pass, x stays in SBUF):
  - DVE: running max over c (7 fp16 TT per b), eq per channel-pair
    (one [128,4096] is_equal vs broadcast max)
  - PE:  fp16 pooling contraction with the innermost W-pair reduction
    folded into PSUM accumulation ([128,512] PSUM tile), then one
    fused XY reduce per quad
  - two tiny AllGathers (one per batch; the first doubles as the
    collective warm-up)
  - corr = att*adj, split across engines: per-wb scale ops on DVE/ACT
    and one broadcast-strided TT per channel on GpSimd
  - phase-2 tiles triple-buffered so pair chains overlap their stores
  - ACT: u2 = (corr+1)^2 as one [128,4096] Square per channel-pair
  - DVE: o1 = x * u2 as one [128,4096] TT per channel-pair
Outputs are stored fp16 (rel err ~5e-4); the host upcasts to f32.
"""

import sys

import numpy as np

try:
    import concourse.bass as bass
except ImportError:  # fresh grading dir: concourse lives in the repo checkout
    for p in ("/opt/trn_rl_repo", "/root/.axon_site/_ro/trn_rl_repo"):
        if p not in sys.path:
            sys.path.insert(0, p)
    import concourse.bass as bass

import concourse.bacc as bacc
import concourse.mybir as mybir
import concourse.tile as tile
from concourse.tile import add_dep_helper
from concourse import bass_utils

B, C, D, H, W = 2, 8, 128, 128, 128
POOL = 8          # pooling block edge
PATCH = 16        # fold patch edge
G = D // PATCH    # 8 patches per spatial dim
NCORES = 8
DL = D // NCORES  # 16 d-slices per core
PD = DL // POOL   # 2 pooled kd-blocks per core
CQ = 4            # channels per gather quad

F32 = mybir.dt.float32
F16 = mybir.dt.float16

OUT_DT = F16          # output store dtype (fp16 halves store traffic)

# static engine split for phase-2 work (tuned from traces)
# corr: "dvet"/"gpst" = one broadcast-strided TT per channel on DVE/GpSimd;
#       "act" = 8 per-wb copy-with-scale ops on the scalar engine
CORR_ENG = {0: "act", 1: "gpst", 2: "act", 3: "dve",
            4: "gpst", 5: "dve", 6: "dve", 7: "gpst"}
O1T_ENG = {0: "gps", 1: "dve", 2: "dve", 3: "dve"}
EQ_ENG = {0: "dve", 1: "dve", 2: "dve", 3: "dve"}

_CACHE = {}


def _build_nc():
    nc = bacc.Bacc("TRN2", target_bir_lowering=False, debug=False,
                   num_devices=NCORES)

    xs = nc.dram_tensor("xs", [B, C, DL, H, W], F16, kind="ExternalInput").ap()
    # attp[a, b, c, wb] = att[b, c, (core*8+a)*8 + wb] / 512
    attp = nc.dram_tensor("attp", [POOL, B, C, G], F32,
                          kind="ExternalInput").ap()
    # pooling lhsT halves: pmat[h][(d,a), 16h + (kd,a')] = 1 iff kd==d//8, a'==a
    pmat = nc.dram_tensor("pmat", [2, 128, 2 * PATCH], F16,
                          kind="ExternalInput").ap()
    o1 = nc.dram_tensor("o1", [B, C, DL, H, W], OUT_DT, kind="ExternalOutput").ap()
    o2 = nc.dram_tensor("o2", [B, C, DL, H, W], OUT_DT, kind="ExternalOutput").ap()

    QS = CQ * PATCH * PATCH  # 1024: free size of one gathered quad row

    with tile.TileContext(nc) as tc:
        with (
            tc.tile_pool(name="big", bufs=1) as big,
            tc.tile_pool(name="xp", bufs=8) as xp,
            tc.tile_pool(name="p1", bufs=2) as p1,
            tc.tile_pool(name="p2", bufs=3) as p2,
            tc.tile_pool(name="psum", bufs=2, space="PSUM") as pp,
            tc.tile_pool(name="dram", bufs=1, space="DRAM") as dram,
        ):
            P2m = big.tile([128, 2, 2 * PATCH], F16, name="P2m")
            A_all = big.tile([128, B * C * G], F32, name="A_all")
            AdjR = {(b, q): big.tile([128, QS], F16, name=f"AdjR{b}{q}")
                    for b in range(B) for q in range(2)}

            nc.scalar.dma_start(out=P2m, in_=pmat.transpose([1, 0, 2]))
            # replicate attp over the d partition index (stride-0 -> SWDGE)
            arep = bass.AP(tensor=attp.tensor, offset=attp.offset,
                           ap=[[0, DL], [B * C * G, POOL], [1, B * C * G]])
            nc.gpsimd.dma_start(out=A_all, in_=arep)
            # fp16 copy of the same (cast during SWDGE) for 16-bit TT corr
            A16 = big.tile([128, B * C * G], F16, name="A16")
            nc.gpsimd.dma_start(out=A16, in_=arep)

            adj_in = {b: dram.tile([PD, 2, CQ, PATCH, PATCH], F32,
                                   name=f"adj_in{b}")
                      for b in range(B)}
            adj_gat = {b: dram.tile([NCORES, PD, 2, CQ, PATCH, PATCH], F32,
                                    name=f"adj_gat{b}",
                                    addr_space="Shared")
                      for b in range(B)}

            xt = {}
            # ---- phase 1: argmax one-hot + pooled counts (per b) ----
            for b in range(B):
                for c2 in range(4):
                    t = xp.tile([128, 2, PATCH * W], F16, name=f"x{b}{c2}",
                                tag="x")
                    xt[(b, c2)] = t
                    nc.sync.dma_start(
                        out=t,
                        in_=xs[b, 2 * c2:2 * c2 + 2].rearrange(
                            "c d (a k) w -> (d a) c (k w)", a=POOL))
                # running max over the 8 channels
                m_prev = None
                for c in range(1, C):
                    m_new = p1.tile([128, PATCH * W], F16, name=f"m{b}{c}",
                                    tag="m")
                    a_in = xt[(b, 0)][:, 0, :] if c == 1 else m_prev
                    nc.vector.tensor_max(m_new, a_in, xt[(b, c // 2)][:, c % 2, :])
                    m_prev = m_new
                Mx = m_prev
                # broadcast AP of Mx over the channel-pair dim (stride 0)
                Mx2 = bass.AP(tensor=Mx.tensor, offset=Mx.offset,
                              ap=[list(Mx.ap[0]), [0, 2], [1, PATCH * W]])

                # pooling matmuls: wi-pairs folded into PSUM accumulation;
                # psum col (k, w8, wi2) = sum_j eq[.., k, w8*8 + 2j + wi2]
                ps = pp.tile([128, 32 * PATCH], F32, name=f"ps{b}", tag="ps")
                for q in range(2):
                    for cl2 in range(2):
                        c2 = q * 2 + cl2
                        eqp = p1.tile([128, 2, PATCH * W], F16,
                                      name=f"eq{b}{c2}", tag="eq", bufs=4)
                        eq_e = (nc.gpsimd if EQ_ENG[c2] == "gps"
                                else nc.vector)
                        eq_e.tensor_tensor(eqp, xt[(b, c2)], Mx2,
                                           op=mybir.AluOpType.is_equal)
                        for half in range(2):
                            eqv = eqp[:, half, :].rearrange(
                                "p (k w8 wi) -> p k w8 wi", k=16, w8=16)
                            for j in range(4):
                                nc.tensor.matmul(
                                    ps[c2 * 32:(c2 + 1) * 32, :],
                                    lhsT=P2m[:, half, :],
                                    rhs=eqv[:, :, :, 2 * j:2 * j + 2],
                                    start=(half == 0 and j == 0),
                                    stop=(half == 1 and j == 3),
                                    tile_position=(0, c2 * 32))
                    # fused pooled reduce for this quad: sum (ki, wi2)
                    A2 = p1.tile([64, 2, PATCH], F32, name=f"a2{b}{q}", tag="a2")
                    last_ph1_dve = nc.vector.reduce_sum(
                        A2, ps[64 * q:64 * (q + 1)].rearrange(
                            "p (k2 ki w8 wi2) -> p k2 w8 ki wi2",
                            k2=2, ki=8, w8=16),
                        axis=mybir.AxisListType.XY)
                    # A2[(cl,kd,a), (k2,w8)] -> adj_in[b][kd, q, cl, 2a+k2, w8]
                    for cl in range(CQ):
                        adj_out = bass.AP(
                            tensor=adj_in[b].tensor,
                            offset=adj_in[b].offset + q * 1024 + cl * 256,
                            ap=[[2 * CQ * 256, PD], [2 * PATCH, POOL],
                                [1, 2 * PATCH]])
                        nc.scalar.dma_start(
                            out=adj_out, in_=A2[cl * PATCH:(cl + 1) * PATCH])
                nc.gpsimd.collective_compute(
                    "AllGather", mybir.AluOpType.bypass,
                    replica_groups=[list(range(NCORES))],
                    ins=[adj_in[b].opt()], outs=[adj_gat[b].opt()])
                for q in range(2):
                    # gathered [core, kd, q, cl, kh, kw]; rows (core,kd) x a
                    repg = bass.AP(tensor=adj_gat[b].tensor,
                                   offset=adj_gat[b].offset + q * QS,
                                   ap=[[2 * QS, DL], [0, POOL], [1, QS]])
                    nc.gpsimd.dma_start(out=AdjR[(b, q)], in_=repg)

            # ---- phase 2: corr / u2 / o1 per (b,c), pair-coalesced stores.
            # Two sub-passes per batch (all corr first, then Square/o1/store)
            # so no engine's queue head-of-line blocks on another engine's
            # unfinished corr half. ----
            first_ph2_dve = None
            for b in range(B):
                cps = {}
                for c2 in range(4):
                    cpair = p2.tile([128, 2, PATCH, G, PATCH], OUT_DT,
                                    name=f"cp{b}{c2}", tag="cp", bufs=5)
                    cps[c2] = cpair
                    for half in range(2):
                        c = 2 * c2 + half
                        q, cl = c // CQ, c % CQ
                        Rc = AdjR[(b, q)][:, cl * 256:(cl + 1) * 256].rearrange(
                            "p (k wi) -> p k wi", k=PATCH)
                        corr = cpair[:, half]
                        if CORR_ENG[c] == "act":
                            for wb in range(G):
                                acol = A_all[:, (b * C + c) * G + wb:
                                             (b * C + c) * G + wb + 1]
                                nc.scalar.mul(corr[:, :, wb, :], Rc, acol)
                        elif CORR_ENG[c] == "dve":
                            for wb in range(G):
                                acol = A_all[:, (b * C + c) * G + wb:
                                             (b * C + c) * G + wb + 1]
                                ins = nc.vector.tensor_scalar_mul(
                                    corr[:, :, wb, :], Rc, acol)
                                if first_ph2_dve is None:
                                    first_ph2_dve = ins
                                    add_dep_helper(ins.ins, last_ph1_dve.ins,
                                                   False, "ph1 DVE first")
                        else:
                            # one op: corr[p,k,wb,wi] = adj[p,k,wi]*att[p,wb]
                            # via stride-0 broadcast APs on both operands
                            ab = AdjR[(b, q)]
                            Ra = bass.AP(
                                tensor=ab.tensor,
                                offset=ab.offset + cl * 256,
                                ap=[list(ab.ap[0]), [PATCH, PATCH],
                                    [0, G], [1, PATCH]])
                            Aa = bass.AP(
                                tensor=A16.tensor,
                                offset=A16.offset + (b * C + c) * G,
                                ap=[list(A16.ap[0]), [0, PATCH],
                                    [1, G], [0, PATCH]])
                            e = (nc.vector if CORR_ENG[c] == "dvet"
                                 else nc.gpsimd)
                            ins = e.tensor_tensor(corr, Ra, Aa,
                                                  op=mybir.AluOpType.mult)
                            if CORR_ENG[c] == "dvet" and first_ph2_dve is None:
                                first_ph2_dve = ins
                                add_dep_helper(ins.ins, last_ph1_dve.ins,
                                               False, "ph1 DVE first")
                    # o2 only needs corr: store it now so the traffic drains
                    # during the Square/o1 pass instead of at the very end
                    ov2 = o2[b, 2 * c2:2 * c2 + 2].rearrange(
                        "c d (a k) w -> (d a) c (k w)", a=POOL)
                    nc.scalar.dma_start(
                        out=ov2, in_=cpair.rearrange("p c a g k -> p c (a g k)"))
                for c2 in range(4):
                    cpair = cps[c2]
                    opair = p2.tile([128, 2, PATCH * W], OUT_DT,
                                    name=f"op{b}{c2}", tag="op")
                    u2 = p2.tile([128, 2, PATCH * W], F16, name=f"u2{b}{c2}",
                                 tag="u2")
                    nc.scalar.activation(
                        u2.rearrange("p c f -> p (c f)"),
                        cpair.rearrange("p c a g k -> p (c a g k)"),
                        mybir.ActivationFunctionType.Square,
                        bias=1.0, scale=1.0)
                    eng = nc.gpsimd if O1T_ENG[c2] == "gps" else nc.vector
                    ins = eng.tensor_mul(opair, xt[(b, c2)], u2)
                    if O1T_ENG[c2] == "dve" and first_ph2_dve is None:
                        first_ph2_dve = ins
                        add_dep_helper(ins.ins, last_ph1_dve.ins,
                                       False, "ph1 DVE first")
                    ov1 = o1[b, 2 * c2:2 * c2 + 2].rearrange(
                        "c d (a k) w -> (d a) c (k w)", a=POOL)
                    nc.sync.dma_start(
                        out=ov1, in_=opair)

    nc.compile()
    return nc


def _prep_x(x):
    """Round x to fp16 such that the fp16 one-hot (equality vs fp16 max)
    EXACTLY reproduces one_hot(argmax) of the f32 input: non-argmax
    channels that would round to >= the argmax channel's fp16 value are
    clamped one fp16 ulp below it (perturbation <= 1 ulp ~ 5e-4 rel)."""
    am = np.argmax(x, axis=1)
    xh = x.astype(np.float16)
    amv = np.take_along_axis(xh, am[:, None], axis=1)  # [B,1,D,H,W]
    clampv = np.nextafter(amv, np.float16(-np.inf), dtype=np.float16)
    oh = np.arange(C, dtype=np.int64)[None, :, None, None, None] == am[:, None]
    return np.where(oh, amv, np.minimum(xh, clampv))


def _host_inputs(x, attentions):
    """Build per-core input maps from full inputs."""
    xh = _prep_x(x)
    att = attentions[..., 0].astype(np.float32) * np.float32(1.0 / 512.0)
    att_p = att.reshape(B, C, G, G, G)  # [b, c, dp, hp, wp]
    pm = np.zeros((2, 128, 2 * PATCH), dtype=np.float16)
    for h in range(2):
        for d in range(DL):
            for a in range(POOL):
                pm[h, d * POOL + a, 16 * h + (d // POOL) * POOL + a] = 1.0

    in_maps = []
    for core in range(NCORES):
        xsc = np.ascontiguousarray(xh[:, :, core * DL:(core + 1) * DL])
        # attp[a, b, c, wb] = att_p[b, c, core, a, wb]
        attp = np.ascontiguousarray(
            att_p[:, :, core].transpose(2, 0, 1, 3)).astype(np.float32)
        in_maps.append({"xs": xsc, "attp": attp, "pmat": pm})
    return in_maps


def kernel(x, attentions):
    x = np.asarray(x, dtype=np.float32)
    attentions = np.asarray(attentions, dtype=np.float32)

    if "nc" not in _CACHE:
        _CACHE["nc"] = _build_nc()
    nc = _CACHE["nc"]

    in_maps = _host_inputs(x, attentions)
    res = bass_utils.run_bass_kernel_spmd(nc, in_maps,
                                          core_ids=list(range(NCORES)))

    out1 = np.empty((B, C, D, H, W), np.float32)
    out2 = np.empty((B, C, D, H, W), np.float32)
    for core in range(NCORES):
        out1[:, :, core * DL:(core + 1) * DL] = np.asarray(
            res.results[core]["o1"], dtype=np.float32)
        out2[:, :, core * DL:(core + 1) * DL] = np.asarray(
            res.results[core]["o2"], dtype=np.float32)
    return out1, out2


# revision 61
# speedup vs baseline: 1.2836x; 1.2836x over previous
"""Trainium2 Bass kernel for the DPAAUser3D segment-reduce problem.

Computes, for x[B=2,C=8,D=H=W=128] and attentions[B,C,512,1]:
  onehot = one_hot(argmax_c x)                      (per-voxel channel argmax)
  adj    = avgpool_8x8x8(onehot)                    ([B,C,16,16,16], = counts/512)
  corr[b,c,D,H,W] = att[b,c,(D//16*8+H//16)*8+W//16] * adj[b,c,D%16,H%16,W%16]
  out1   = x * (1+corr)^2
  out2   = corr

Sharding: data-parallel over D (16 slices per core, 8 cores); per-core
pooled counts are AllGathered per (batch, channel-quad).

v4: fp16 end-to-end. The host rounds x to fp16 and nudges so the fp16
argmax one-hot EXACTLY matches the f32 argmax one-hot (non-argmax
channels that round to >= the argmax value are clamped one fp16 ulp
below it; perturbation <= 1 ulp ~ 5e-4 rel). Consequences:
  - x HBM load traffic halves (fp16 instead of f32)
  - every DVE tensor_tensor runs in 2x perf mode (16-bit packed)
  - argmax/one-hot is numerically EXACT vs the reference
Pipeline per core (single pass, x stays in SBUF):
  - DVE: running max over c (7 fp16 TT per b), eq per channel-pair
    (one [128,4096] is_equal vs broadcast max)
  - PE:  fp16 pooling contraction with the innermost W-pair reduction
    folded into PSUM accumulation ([128,512] PSUM tile), then one
    fused XY reduce per quad
  - two tiny AllGathers (one per batch; the first doubles as the
    collective warm-up)
  - corr = att*adj, split across engines: per-wb scale ops on DVE/ACT
    and one broadcast-strided TT per channel on GpSimd
  - phase-2 tiles triple-buffered so pair chains overlap their stores
  - ACT: u2 = (corr+1)^2 as one [128,4096] Square per channel-pair
  - DVE: o1 = x * u2 as one [128,4096] TT per channel-pair
Outputs are stored fp16 (rel err ~5e-4); the host upcasts to f32.
"""

import sys

import numpy as np

try:
    import concourse.bass as bass
except ImportError:  # fresh grading dir: concourse lives in the repo checkout
    for p in ("/opt/trn_rl_repo", "/root/.axon_site/_ro/trn_rl_repo"):
        if p not in sys.path:
            sys.path.insert(0, p)
    import concourse.bass as bass

import concourse.bacc as bacc
import concourse.mybir as mybir
import concourse.tile as tile
from concourse.tile import add_dep_helper
from concourse import bass_utils

B, C, D, H, W = 2, 8, 128, 128, 128
POOL = 8          # pooling block edge
PATCH = 16        # fold patch edge
G = D // PATCH    # 8 patches per spatial dim
NCORES = 8
DL = D // NCORES  # 16 d-slices per core
PD = DL // POOL   # 2 pooled kd-blocks per core
CQ = 4            # channels per gather quad

F32 = mybir.dt.float32
F16 = mybir.dt.float16

OUT_DT = F16          # output store dtype (fp16 halves store traffic)

# static engine split for phase-2 work (tuned from traces)
# corr: "dvet"/"gpst" = one broadcast-strided TT per channel on DVE/GpSimd;
#       "act" = 8 per-wb copy-with-scale ops on the scalar engine
CORR_ENG = {0: "act", 1: "gpst", 2: "act", 3: "dve",
            4: "gpst", 5: "dve", 6: "dve", 7: "gpst"}
O1T_ENG = {0: "gps", 1: "dve", 2: "dve", 3: "dve"}
EQ_ENG = {0: "dve", 1: "dve", 2: "dve", 3: "dve"}

_CACHE = {}


def _build_nc():
    nc = bacc.Bacc("TRN2", target_bir_lowering=False, debug=False,
                   num_devices=NCORES)

    xs = nc.dram_tensor("xs", [B, C, DL, H, W], F16, kind="ExternalInput").ap()
    # attp[a, b, c, wb] = att[b, c, (core*8+a)*8 + wb] / 512
    attp = nc.dram_tensor("attp", [POOL, B, C, G], F32,
                          kind="ExternalInput").ap()
    # pooling lhsT halves: pmat[h][(d,a), 16h + (kd,a')] = 1 iff kd==d//8, a'==a
    pmat = nc.dram_tensor("pmat", [2, 128, 2 * PATCH], F16,
                          kind="ExternalInput").ap()
    o1 = nc.dram_tensor("o1", [B, C, DL, H, W], OUT_DT, kind="ExternalOutput").ap()
    o2 = nc.dram_tensor("o2", [B, C, DL, H, W], OUT_DT, kind="ExternalOutput").ap()

    QS = CQ * PATCH * PATCH  # 1024: free size of one gathered quad row

    with tile.TileContext(nc) as tc:
        with (
            tc.tile_pool(name="big", bufs=1) as big,
            tc.tile_pool(name="xp", bufs=8) as xp,
            tc.tile_pool(name="p1", bufs=2) as p1,
            tc.tile_pool(name="p2", bufs=3) as p2,
            tc.tile_pool(name="psum", bufs=2, space="PSUM") as pp,
            tc.tile_pool(name="dram", bufs=1, space="DRAM") as dram,
        ):
            P2m = big.tile([128, 2, 2 * PATCH], F16, name="P2m")
            A_all = big.tile([128, B * C * G], F32, name="A_all")
            AdjR = {(b, q): big.tile([128, QS], F16, name=f"AdjR{b}{q}")
                    for b in range(B) for q in range(2)}

            nc.scalar.dma_start(out=P2m, in_=pmat.transpose([1, 0, 2]))
            # replicate attp over the d partition index (stride-0 -> SWDGE)
            arep = bass.AP(tensor=attp.tensor, offset=attp.offset,
                           ap=[[0, DL], [B * C * G, POOL], [1, B * C * G]])
            nc.gpsimd.dma_start(out=A_all, in_=arep)
            # fp16 copy of the same (cast during SWDGE) for 16-bit TT corr
            A16 = big.tile([128, B * C * G], F16, name="A16")
            nc.gpsimd.dma_start(out=A16, in_=arep)

            adj_in = {b: dram.tile([PD, 2, CQ, PATCH, PATCH], F32,
                                   name=f"adj_in{b}")
                      for b in range(B)}
            adj_gat = {b: dram.tile([NCORES, PD, 2, CQ, PATCH, PATCH], F32,
                                    name=f"adj_gat{b}",
                                    addr_space="Shared")
                      for b in range(B)}

            xt = {}
            # ---- phase 1: argmax one-hot + pooled counts (per b) ----
            for b in range(B):
                for c2 in range(4):
                    t = xp.tile([128, 2, PATCH * W], F16, name=f"x{b}{c2}",
                                tag="x")
                    xt[(b, c2)] = t
                    nc.sync.dma_start(
                        out=t,
                        in_=xs[b, 2 * c2:2 * c2 + 2].rearrange(
                            "c d (a k) w -> (d a) c (k w)", a=POOL))
                # running max over the 8 channels
                m_prev = None
                for c in range(1, C):
                    m_new = p1.tile([128, PATCH * W], F16, name=f"m{b}{c}",
                                    tag="m")
                    a_in = xt[(b, 0)][:, 0, :] if c == 1 else m_prev
                    nc.vector.tensor_max(m_new, a_in, xt[(b, c // 2)][:, c % 2, :])
                    m_prev = m_new
                Mx = m_prev
                # broadcast AP of Mx over the channel-pair dim (stride 0)
                Mx2 = bass.AP(tensor=Mx.tensor, offset=Mx.offset,
                              ap=[list(Mx.ap[0]), [0, 2], [1, PATCH * W]])

                # pooling matmuls: wi-pairs folded into PSUM accumulation;
                # psum col (k, w8, wi2) = sum_j eq[.., k, w8*8 + 2j + wi2]
                ps = pp.tile([128, 32 * PATCH], F32, name=f"ps{b}", tag="ps")
                for q in range(2):
                    for cl2 in range(2):
                        c2 = q * 2 + cl2
                        eqp = p1.tile([128, 2, PATCH * W], F16,
                                      name=f"eq{b}{c2}", tag="eq", bufs=4)
                        eq_e = (nc.gpsimd if EQ_ENG[c2] == "gps"
                                else nc.vector)
                        eq_e.tensor_tensor(eqp, xt[(b, c2)], Mx2,
                                           op=mybir.AluOpType.is_equal)
                        for half in range(2):
                            eqv = eqp[:, half, :].rearrange(
                                "p (k w8 wi) -> p k w8 wi", k=16, w8=16)
                            for j in range(4):
                                nc.tensor.matmul(
                                    ps[c2 * 32:(c2 + 1) * 32, :],
                                    lhsT=P2m[:, half, :],
                                    rhs=eqv[:, :, :, 2 * j:2 * j + 2],
                                    start=(half == 0 and j == 0),
                                    stop=(half == 1 and j == 3),
                                    tile_position=(0, c2 * 32))
                    # fused pooled reduce for this quad: sum (ki, wi2)
                    A2 = p1.tile([64, 2, PATCH], F32, name=f"a2{b}{q}", tag="a2")
                    last_ph1_dve = nc.vector.reduce_sum(
                        A2, ps[64 * q:64 * (q + 1)].rearrange(
                            "p (k2 ki w8 wi2) -> p k2 w8 ki wi2",
                            k2=2, ki=8, w8=16),
                        axis=mybir.AxisListType.XY)
                    # A2[(cl,kd,a), (k2,w8)] -> adj_in[b][kd, q, cl, 2a+k2, w8]
                    for cl in range(CQ):
                        adj_out = bass.AP(
                            tensor=adj_in[b].tensor,
                            offset=adj_in[b].offset + q * 1024 + cl * 256,
                            ap=[[2 * CQ * 256, PD], [2 * PATCH, POOL],
                                [1, 2 * PATCH]])
                        nc.scalar.dma_start(
                            out=adj_out, in_=A2[cl * PATCH:(cl + 1) * PATCH])
                nc.gpsimd.collective_compute(
                    "AllGather", mybir.AluOpType.bypass,
                    replica_groups=[list(range(NCORES))],
                    ins=[adj_in[b].opt()], outs=[adj_gat[b].opt()])
                for q in range(2):
                    # gathered [core, kd, q, cl, kh, kw]; rows (core,kd) x a
                    repg = bass.AP(tensor=adj_gat[b].tensor,
                                   offset=adj_gat[b].offset + q * QS,
                                   ap=[[2 * QS, DL], [0, POOL], [1, QS]])
                    nc.gpsimd.dma_start(out=AdjR[(b, q)], in_=repg)

            # ---- phase 2: corr / u2 / o1 per (b,c), pair-coalesced stores.
            # Two sub-passes per batch (all corr first, then Square/o1/store)
            # so no engine's queue head-of-line blocks on another engine's
            # unfinished corr half. ----
            first_ph2_dve = None
            for b in range(B):
                cps = {}
                for c2 in range(4):
                    cpair = p2.tile([128, 2, PATCH, G, PATCH], OUT_DT,
                                    name=f"cp{b}{c2}", tag="cp", bufs=4)
                    cps[c2] = cpair
                    for half in range(2):
                        c = 2 * c2 + half
                        q, cl = c // CQ, c % CQ
                        Rc = AdjR[(b, q)][:, cl * 256:(cl + 1) * 256].rearrange(
                            "p (k wi) -> p k wi", k=PATCH)
                        corr = cpair[:, half]
                        if CORR_ENG[c] == "act":
                            for wb in range(G):
                                acol = A_all[:, (b * C + c) * G + wb:
                                             (b * C + c) * G + wb + 1]
                                nc.scalar.mul(corr[:, :, wb, :], Rc, acol)
                        elif CORR_ENG[c] == "dve":
                            for wb in range(G):
                                acol = A_all[:, (b * C + c) * G + wb:
                                             (b * C + c) * G + wb + 1]
                                ins = nc.vector.tensor_scalar_mul(
                                    corr[:, :, wb, :], Rc, acol)
                                if first_ph2_dve is None:
                                    first_ph2_dve = ins
                                    add_dep_helper(ins.ins, last_ph1_dve.ins,
                                                   False, "ph1 DVE first")
                        else:
                            # one op: corr[p,k,wb,wi] = adj[p,k,wi]*att[p,wb]
                            # via stride-0 broadcast APs on both operands
                            ab = AdjR[(b, q)]
                            Ra = bass.AP(
                                tensor=ab.tensor,
                                offset=ab.offset + cl * 256,
                                ap=[list(ab.ap[0]), [PATCH, PATCH],
                                    [0, G], [1, PATCH]])
                            Aa = bass.AP(
                                tensor=A16.tensor,
                                offset=A16.offset + (b * C + c) * G,
                                ap=[list(A16.ap[0]), [0, PATCH],
                                    [1, G], [0, PATCH]])
                            e = (nc.vector if CORR_ENG[c] == "dvet"
                                 else nc.gpsimd)
                            ins = e.tensor_tensor(corr, Ra, Aa,
                                                  op=mybir.AluOpType.mult)
                            if CORR_ENG[c] == "dvet" and first_ph2_dve is None:
                                first_ph2_dve = ins
                                add_dep_helper(ins.ins, last_ph1_dve.ins,
                                               False, "ph1 DVE first")
                    # o2 only needs corr: store it now so the traffic drains
                    # during the Square/o1 pass instead of at the very end
                    ov2 = o2[b, 2 * c2:2 * c2 + 2].rearrange(
                        "c d (a k) w -> (d a) c (k w)", a=POOL)
                    nc.scalar.dma_start(
                        out=ov2, in_=cpair.rearrange("p c a g k -> p c (a g k)"))
                for c2 in range(4):
                    cpair = cps[c2]
                    opair = p2.tile([128, 2, PATCH * W], OUT_DT,
                                    name=f"op{b}{c2}", tag="op", bufs=4)
                    u2 = p2.tile([128, 2, PATCH * W], F16, name=f"u2{b}{c2}",
                                 tag="u2")
                    nc.scalar.activation(
                        u2.rearrange("p c f -> p (c f)"),
                        cpair.rearrange("p c a g k -> p (c a g k)"),
                        mybir.ActivationFunctionType.Square,
                        bias=1.0, scale=1.0)
                    eng = nc.gpsimd if O1T_ENG[c2] == "gps" else nc.vector
                    ins = eng.tensor_mul(opair, xt[(b, c2)], u2)
                    if O1T_ENG[c2] == "dve" and first_ph2_dve is None:
                        first_ph2_dve = ins
                        add_dep_helper(ins.ins, last_ph1_dve.ins,
                                       False, "ph1 DVE first")
                    ov1 = o1[b, 2 * c2:2 * c2 + 2].rearrange(
                        "c d (a k) w -> (d a) c (k w)", a=POOL)
                    nc.sync.dma_start(
                        out=ov1, in_=opair)

    nc.compile()
    return nc


def _prep_x(x):
    """Round x to fp16 such that the fp16 one-hot (equality vs fp16 max)
    EXACTLY reproduces one_hot(argmax) of the f32 input: non-argmax
    channels that would round to >= the argmax channel's fp16 value are
    clamped one fp16 ulp below it (perturbation <= 1 ulp ~ 5e-4 rel)."""
    am = np.argmax(x, axis=1)
    xh = x.astype(np.float16)
    amv = np.take_along_axis(xh, am[:, None], axis=1)  # [B,1,D,H,W]
    clampv = np.nextafter(amv, np.float16(-np.inf), dtype=np.float16)
    oh = np.arange(C, dtype=np.int64)[None, :, None, None, None] == am[:, None]
    return np.where(oh, amv, np.minimum(xh, clampv))


def _host_inputs(x, attentions):
    """Build per-core input maps from full inputs."""
    xh = _prep_x(x)
    att = attentions[..., 0].astype(np.float32) * np.float32(1.0 / 512.0)
    att_p = att.reshape(B, C, G, G, G)  # [b, c, dp, hp, wp]
    pm = np.zeros((2, 128, 2 * PATCH), dtype=np.float16)
    for h in range(2):
        for d in range(DL):
            for a in range(POOL):
                pm[h, d * POOL + a, 16 * h + (d // POOL) * POOL + a] = 1.0

    in_maps = []
    for core in range(NCORES):
        xsc = np.ascontiguousarray(xh[:, :, core * DL:(core + 1) * DL])
        # attp[a, b, c, wb] = att_p[b, c, core, a, wb]
        attp = np.ascontiguousarray(
            att_p[:, :, core].transpose(2, 0, 1, 3)).astype(np.float32)
        in_maps.append({"xs": xsc, "attp": attp, "pmat": pm})
    return in_maps


def kernel(x, attentions):
    x = np.asarray(x, dtype=np.float32)
    attentions = np.asarray(attentions, dtype=np.float32)

    if "nc" not in _CACHE:
        _CACHE["nc"] = _build_nc()
    nc = _CACHE["nc"]

    in_maps = _host_inputs(x, attentions)
    res = bass_utils.run_bass_kernel_spmd(nc, in_maps,
                                          core_ids=list(range(NCORES)))

    out1 = np.empty((B, C, D, H, W), np.float32)
    out2 = np.empty((B, C, D, H, W), np.float32)
    for core in range(NCORES):
        out1[:, :, core * DL:(core + 1) * DL] = np.asarray(
            res.results[core]["o1"], dtype=np.float32)
        out2[:, :, core * DL:(core + 1) * DL] = np.asarray(
            res.results[core]["o2"], dtype=np.float32)
    return out1, out2
